# revision 4
# baseline (speedup 1.0000x reference)
"""Multi-head causal self-attention (B=2, N=4096, C=512, H=8, D=64) on 8 TRN2 cores.

Sharding: core = b*4 + g  (b = batch 0..1, g = head-group 0..3, 2 heads each).

v2 restructure vs baseline:
- AV matmuls use P as the stationary operand: out' [q, v] per 128-q chunk,
  free dim 65 (64 v-dims + ones column for the softmax denominator) instead
  of 512 -> AV PE cycles halve.
- Softmax normalization becomes a per-partition scalar multiply (DVE), no
  gpsimd partition_broadcast.
- The [q, v] -> [v, q] layout flip for the projection uses the DMA XBAR
  transpose (SBUF->SBUF, off-engine).
- exp splits between ACT (exact, Exp activation) and DVE (Schraudolph:
  one tensor_scalar computing i16 = rint(A*s + B) whose bit pattern IS the
  bf16 of exp(s); ~2% deterministic wiggle, mostly cancelled by the
  consistent denominator).
- tri-mask multiplies run on Pool (gpsimd, SBUF-only engine).
"""

import os

import numpy as np
import ml_dtypes

_CACHE: dict = {}
LAST_RESULTS = None

B, C = 2, 512
H, D = 8, 64
N = 4096
NQT = 8          # q tiles of 512
NKB = 32         # key blocks of 128
QT = 512
KB = 128

# Schraudolph bf16-exp constants (i16 = rint(A*s + Bc), RNE convert verified)
SCH_A = 2.0**7 / float(np.log(2.0))
SCH_B = 127.0 * 2.0**7 - 0.0573 * 2.0**7

# fraction control: full-pair units with (index % SCH_MOD == SCH_PHASE) go to DVE
SCH_MOD = 3
SCH_PHASE = 1


def _build():
    import concourse.bass as bass
    import concourse.bacc as bacc
    import concourse.mybir as mybir
    import concourse.tile as tile

    dt = mybir.dt
    bf = dt.bfloat16
    f32 = dt.float32
    i16 = dt.int16
    Exp = mybir.ActivationFunctionType.Exp
    Copy = mybir.ActivationFunctionType.Copy
    Alu = mybir.AluOpType

    debug = bool(os.environ.get("KERNEL_DEBUG"))
    f8 = dt.float8e4
    DR = mybir.MatmulPerfMode.DoubleRow
    nc = bacc.Bacc("TRN2", target_bir_lowering=False)
    xth = nc.dram_tensor("xth", [C, N], f8, kind="ExternalInput")
    xtl = nc.dram_tensor("xtl", [C, N], f8, kind="ExternalInput")
    wq8 = nc.dram_tensor("wq8", [2, C, 128], f8, kind="ExternalInput")
    wk8 = nc.dram_tensor("wk8", [2, C, 128], f8, kind="ExternalInput")
    wv8 = nc.dram_tensor("wv8", [2, C, 128], f8, kind="ExternalInput")
    wp = nc.dram_tensor("wp", [128, C], bf, kind="ExternalInput")
    tri = nc.dram_tensor("tri", [128, 128], bf, kind="ExternalInput")
    dmask = nc.dram_tensor("dmask", [128, 640], bf, kind="ExternalInput")
    yt = nc.dram_tensor("yt", [C, N], f32, kind="ExternalOutput")
    if debug:
        d_qT = nc.dram_tensor("d_qT", [128, N], bf, kind="ExternalOutput")
        d_kT = nc.dram_tensor("d_kT", [128, N], bf, kind="ExternalOutput")
        d_v = nc.dram_tensor("d_v", [128, NKB, 130], bf, kind="ExternalOutput")
        d_pf = nc.dram_tensor("d_pf", [128, 1024], bf, kind="ExternalOutput")
        d_psO = nc.dram_tensor("d_psO", [128, 4, 128], f32, kind="ExternalOutput")
        d_on = nc.dram_tensor("d_on", [128, 4, 128], bf, kind="ExternalOutput")
        d_ot = nc.dram_tensor("d_ot", [128, 512], bf, kind="ExternalOutput")

    with tile.TileContext(nc) as tc:
        with (
            tc.tile_pool(name="persist", bufs=1) as pp,
            tc.tile_pool(name="pf", bufs=8) as pf_pool,      # P tiles (pairs)
            tc.tile_pool(name="pd", bufs=3) as pd_pool,      # P tiles (diag)
            tc.tile_pool(name="on", bufs=2) as on_pool,      # [q,v] normalized
            tc.tile_pool(name="ot", bufs=2) as ot_pool,      # transposed [v,q]
            tc.tile_pool(name="rc", bufs=2) as rc_pool,      # reciprocals
            tc.tile_pool(name="yo", bufs=3) as yo_pool,      # y staging
            tc.tile_pool(name="ps_s", bufs=3, space="PSUM") as ps_s,
            tc.tile_pool(name="ps_o", bufs=1, space="PSUM") as ps_o,
            tc.tile_pool(name="ps_y", bufs=1, space="PSUM") as ps_y,
        ):
            xt_hi = pp.tile([128, 4, N], f8)
            xt_lo = pp.tile([128, 4, N], f8)
            wq_sb = pp.tile([128, 2, 4, 128], f8)
            wk_sb = pp.tile([128, 2, 4, 128], f8)
            wv_sb = pp.tile([128, 2, 4, 128], f8)
            wp_sb = pp.tile([128, C], bf)
            tri_sb = pp.tile([128, 128], bf)
            qT = pp.tile([128, N], bf)
            kT = pp.tile([128, N], bf)
            v_sb = pp.tile([128, NKB, 130], bf)

            nc.gpsimd.dma_start(out=wq_sb[:, :, :, :], in_=wq8.rearrange("s (c p) f -> p s c f", p=128))
            nc.gpsimd.dma_start(out=wk_sb[:, :, :, :], in_=wk8.rearrange("s (c p) f -> p s c f", p=128))
            nc.gpsimd.dma_start(out=wv_sb[:, :, :, :], in_=wv8.rearrange("s (c p) f -> p s c f", p=128))
            nc.gpsimd.dma_start(out=wp_sb, in_=wp[:, :])
            nc.gpsimd.dma_start(out=tri_sb, in_=tri[:, :])
            dmask_sb = pp.tile([128, 640], bf)
            nc.gpsimd.dma_start(out=dmask_sb, in_=dmask[:, :])
            nc.vector.memset(v_sb, 1.0)
            # trigger the Exp act-table load early, overlapped with input DMAs
            warm = pp.tile([128, 1], f32)
            nc.vector.memset(warm, 0.0)
            nc.scalar.activation(warm, warm, Exp)

            xth_re = xth.rearrange("(c p) n -> p c n", p=128)
            xtl_re = xtl.rearrange("(c p) n -> p c n", p=128)
            UNSCALE = 1.0 / 64.0

            # -------- QKV phase: fp8 DoubleRow, 3-term hi/lo compensation --
            def pa_qk(n, dst, wsb, with_dma):
                def piece():
                    if with_dma:
                        nc.sync.dma_start(
                            out=xt_hi[:, :, QT * n:QT * (n + 1)],
                            in_=xth_re[:, :, QT * n:QT * (n + 1)],
                        )
                        nc.sync.dma_start(
                            out=xt_lo[:, :, QT * n:QT * (n + 1)],
                            in_=xtl_re[:, :, QT * n:QT * (n + 1)],
                        )
                    ps = ps_s.tile([128, 1024], f32, tag="s", name=f"pa_{n}")
                    pq = ps[:, 0:512]
                    terms = [(0, xt_hi), (0, xt_lo), (1, xt_hi)]
                    nmm = 0
                    for s, xt8 in terms:
                        for cp in range(2):
                            nc.tensor.matmul(
                                pq,
                                wsb[:, s, 2 * cp:2 * cp + 2, :],
                                xt8[:, 2 * cp:2 * cp + 2, QT * n:QT * (n + 1)],
                                start=(nmm == 0),
                                stop=(nmm == 5),
                                perf_mode=DR,
                            )
                            nmm += 1
                    nc.scalar.activation(dst[:, QT * n:QT * (n + 1)], pq, Copy, scale=UNSCALE)
                return piece

            def pa_v(n):
                # 4 kb blocks' V in one psum tile, one batched copy out
                def piece():
                    ps = ps_s.tile([128, 1024], f32, tag="s", name=f"pav_{n}")
                    for j in range(4):
                        kb = 4 * n + j
                        pv = ps[:, 128 * j:128 * (j + 1)]
                        terms = [(xt_hi, 0), (xt_lo, 0), (xt_hi, 1)]
                        nmm = 0
                        for xt8, s in terms:
                            for cp in range(2):
                                nc.tensor.matmul(
                                    pv,
                                    xt8[:, 2 * cp:2 * cp + 2, KB * kb:KB * (kb + 1)],
                                    wv_sb[:, s, 2 * cp:2 * cp + 2, :],
                                    start=(nmm == 0),
                                    stop=(nmm == 5),
                                    perf_mode=DR,
                                )
                                nmm += 1
                    nc.scalar.activation(
                        v_sb[:, 4 * n:4 * n + 4, :]
                        .rearrange("p k (h j) -> p k h j", h=2)[:, :, :, 0:64],
                        ps[:, 0:512].rearrange("p (k h j) -> p k h j", k=4, h=2),
                        Copy,
                        scale=UNSCALE,
                    )
                return piece

            def phase_a_pieces(n):
                return [
                    pa_qk(n, qT, wq_sb, True),
                    pa_qk(n, kT, wk_sb, False),
                    pa_v(n),
                ]

            # diag slot layout keeps every matmul inside one 2KB PSUM bank:
            # r1 -> [0:384], r3 -> [384:512] (bank 0), r2 -> [512:768] (bank 1)
            offs = (0, 512, 384)
            wid = (384, 256, 128)

            psO_map = {}
            on_map = {}
            ot_map = {}
            import heapq
            deferred = []
            seq_counter = [0]

            def defer(due, fn):
                heapq.heappush(deferred, (due, seq_counter[0], fn))
                seq_counter[0] += 1

            def flush(i):
                while deferred and deferred[0][0] <= i:
                    heapq.heappop(deferred)[2]()

            def get_psO(qt, h):
                key = (qt, h)
                if key not in psO_map:
                    psO_map[key] = ps_o.tile([128, 4, 128], f32, tag="o", name=f"psO_{qt}_{h}")
                return psO_map[key]

            def get_on(qt):
                if qt not in on_map:
                    on_map[qt] = on_pool.tile([128, 4, 128], bf, tag="on", name=f"on_{qt}")
                return on_map[qt]

            # --- AV' matmuls: P stationary [128 keys, 128 q], V moving [128,65]
            # PSUM has_written semantics: start=True clears the bits for the
            # WHOLE bank, so only the very first matmul of each (qt,h) stream
            # may set it. Later first-writes to other qc ranges overwrite
            # where the bit is unset, which is exactly what we need.
            def make_av(qt, h, contribs):
                # contribs: list of (kb, P_ap_slice_fn(qc) -> AP, qc_range)
                def av():
                    psO = get_psO(qt, h)
                    for kb, pap, qcs in contribs:
                        for qc in qcs:
                            nc.tensor.matmul(
                                psO[:, qc, 0:65],
                                pap(qc),
                                v_sb[:, kb, 65 * h:65 * h + 65],
                                start=(kb == 0 and qc == 0),
                                stop=(kb == 4 * qt + qc),
                                skip_group_check=True,
                            )
                return av

            def make_epilogue(qt, h):
                def epi():
                    psO = psO_map.pop((qt, h))
                    if debug and qt == 2 and h == 0:
                        tmp = rc_pool.tile([128, 4, 128], f32, tag="dbg")
                        nc.vector.tensor_copy(tmp, psO)
                        nc.sync.dma_start(out=d_psO[:, :, :], in_=tmp)
                    rc = rc_pool.tile([128, 4], f32, tag="rc")
                    nc.vector.reciprocal(out=rc, in_=psO[:, :, 64])
                    on = get_on(qt)
                    ot = None
                    if h == 1:
                        ot = ot_pool.tile([128, 512], bf, tag="ot", name=f"ot_{qt}")
                        ot_map[qt] = ot
                        if debug and qt == 2:
                            nc.sync.dma_start(out=d_on[:, :, :], in_=on)
                    for qc in range(4):
                        nc.vector.tensor_scalar(
                            out=on[:, qc, 64 * h:64 * h + 64],
                            in0=psO[:, qc, 0:64],
                            scalar1=rc[:, qc:qc + 1],
                            scalar2=None,
                            op0=mybir.AluOpType.mult,
                        )
                        if h == 1:
                            # transpose each chunk as soon as both halves exist
                            eng = nc.sync if qc % 2 == 0 else nc.scalar
                            eng.dma_start(
                                out=ot[:, 128 * qc:128 * (qc + 1)],
                                in_=on[:, qc, :],
                                transpose=True,
                            )
                    if h == 1:
                        on_map.pop(qt)
                        if debug and qt == 2:
                            nc.sync.dma_start(out=d_ot[:, :], in_=ot)
                return epi

            def make_proj_ob(qt, ob):
                def proj():
                    ot = ot_map[qt]
                    # the last q-tile's projections run in the drain; borrow
                    # the then-idle S pool for double buffering
                    pool = ps_s if qt == NQT - 1 else ps_y
                    tag = "s" if qt == NQT - 1 else "y"
                    psY = pool.tile([128, 512], f32, tag=tag, name=f"psY_{qt}_{ob}")
                    nc.tensor.matmul(
                        psY,
                        wp_sb[:, 128 * ob:128 * (ob + 1)],
                        ot,
                        start=True,
                        stop=True,
                    )
                    y_sb = yo_pool.tile([128, 512], f32, tag="yo")
                    if qt == NQT - 1 and ob % 2 == 1:
                        nc.scalar.activation(y_sb, psY, Copy)
                    else:
                        nc.vector.tensor_copy(y_sb, psY)
                    eng = nc.sync if (qt < NQT - 1 or ob % 2 == 0) else nc.scalar
                    eng.dma_start(
                        out=yt[128 * ob:128 * (ob + 1), QT * qt:QT * (qt + 1)],
                        in_=y_sb,
                    )
                    if ob == 3:
                        ot_map.pop(qt)
                return proj

            ui = 0
            for n in (0, 1):
                for piece in phase_a_pieces(n):
                    piece()
            pa_pending = []
            for qt in range(NQT):
                for piece in pa_pending:
                    piece()
                pa_pending = phase_a_pieces(qt + 2) if qt + 2 < NQT else []
                for h in range(2):
                    b0 = 64 * h
                    pair_idx = 0
                    # ---- full pair units: kb groups of 2 over kb = 0..4qt-1,
                    #      then singleton kb=4qt (block-diagonal, tri-masked)
                    kb = 0
                    while kb <= 4 * qt:
                        w = min(2, 4 * qt + 1 - kb)
                        kbs = list(range(kb, kb + w))
                        is_singleton = kbs[-1] == 4 * qt
                        psS = ps_s.tile([128, 1024], f32, tag="s")
                        for j, kbj in enumerate(kbs):
                            nc.tensor.matmul(
                                psS[:, 512 * j:512 * (j + 1)],
                                kT[b0:b0 + 64, KB * kbj:KB * (kbj + 1)],
                                qT[b0:b0 + 64, QT * qt:QT * (qt + 1)],
                                start=True,
                                stop=True,
                            )
                        Pf = pf_pool.tile([128, 1024], bf, tag="pf")
                        use_sch = (
                            not is_singleton
                            and pair_idx % SCH_MOD == SCH_PHASE
                        )
                        if use_sch:
                            nc.vector.tensor_scalar(
                                out=Pf[:, 0:512 * w].bitcast(i16),
                                in0=psS[:, 0:512 * w],
                                scalar1=SCH_A,
                                scalar2=SCH_B,
                                op0=Alu.mult,
                                op1=Alu.add,
                            )
                        else:
                            nc.scalar.activation(Pf[:, 0:512 * w], psS[:, 0:512 * w], Exp)
                        if is_singleton:
                            j = w - 1
                            nc.gpsimd.tensor_mul(
                                Pf[:, 512 * j:512 * j + 128],
                                Pf[:, 512 * j:512 * j + 128],
                                tri_sb,
                            )
                        if debug and qt == 2 and h == 0 and kb == 0:
                            nc.sync.dma_start(out=d_pf[:, :], in_=Pf)
                        # AV contributions of this unit (bind Pf NOW — late
                        # binding would capture a future unit's tile)
                        contribs = [
                            (kbj, (lambda jj, P: lambda qc: P[:, 512 * jj + 128 * qc:512 * jj + 128 * qc + 128])(j2, Pf), range(4))
                            for j2, kbj in enumerate(kbs)
                        ]
                        flush(ui)
                        last_stream = qt == NQT - 1 and h == 1
                        defer(ui + (1 if last_stream else 4), make_av(qt, h, contribs))
                        if pa_pending:
                            pa_pending.pop(0)()
                        ui += 1
                        kb += w
                        pair_idx += 1
                    # ---- diag unit: r = 1..3 packed [r1|r3|r2]
                    psD = ps_s.tile([128, 1024], f32, tag="s")
                    for r in (1, 2, 3):
                        kbr = 4 * qt + r
                        nc.tensor.matmul(
                            psD[:, offs[r - 1]:offs[r - 1] + wid[r - 1]],
                            kT[b0:b0 + 64, KB * kbr:KB * (kbr + 1)],
                            qT[b0:b0 + 64, QT * qt + 128 * r:QT * qt + 128 * r + wid[r - 1]],
                            start=True,
                            stop=True,
                        )
                    Pd = pd_pool.tile([128, 768], bf, tag="pd")
                    nc.vector.tensor_scalar(
                        out=Pd.bitcast(i16),
                        in0=psD[:, 0:768],
                        scalar1=SCH_A,
                        scalar2=SCH_B,
                        op0=Alu.mult,
                        op1=Alu.add,
                    )
                    nc.vector.tensor_mul(Pd[:, 0:640], Pd[:, 0:640], dmask_sb)
                    contribs = [
                        (4 * qt + r,
                         (lambda rr, P: lambda qc: P[:, offs[rr - 1] + 128 * (qc - rr):offs[rr - 1] + 128 * (qc - rr) + 128])(r, Pd),
                         range(r, 4))
                        for r in (1, 2, 3)
                    ]
                    flush(ui)
                    last_stream = qt == NQT - 1 and h == 1
                    defer(ui + (1 if last_stream else 3), make_av(qt, h, contribs))
                    defer(ui + (2 if last_stream else 4), make_epilogue(qt, h))
                    if h == 1:
                        if qt == NQT - 1:
                            for ob in range(4):
                                defer(ui + 3 + ob, make_proj_ob(qt, ob))
                        else:
                            # land the y-copies in the next qt's h0/h1
                            # boundary hole on ACT
                            for ob in range(4):
                                defer(ui + 5 + 2 * ob, make_proj_ob(qt, ob))
                    if pa_pending:
                        pa_pending.pop(0)()
                    ui += 1
            flush(10 ** 9)
            if debug:
                nc.sync.dma_start(out=d_qT[:, :], in_=qT)
                nc.sync.dma_start(out=d_kT[:, :], in_=kT)
                nc.sync.dma_start(out=d_v[:, :, :], in_=v_sb)

    nc.compile()
    return nc


def kernel(x, w_qkv, w_proj, b_proj):
    global LAST_RESULTS
    from concourse.bass_utils import run_bass_kernel_spmd

    if "nc" not in _CACHE:
        _CACHE["nc"] = _build()
    nc = _CACHE["nc"]

    x = np.asarray(x)
    w_qkv = np.asarray(w_qkv)
    w_proj = np.asarray(w_proj)
    b_proj = np.asarray(b_proj)
    bf16 = ml_dtypes.bfloat16
    e4m3 = ml_dtypes.float8_e4m3fn
    scale = D ** -0.5

    def split8(a):
        hi = a.astype(e4m3)
        lo = (a - hi.astype(np.float32)).astype(e4m3)
        return np.stack([hi, lo])

    tri = np.triu(np.ones((128, 128), np.float32)).astype(bf16)
    trif = np.triu(np.ones((128, 128), np.float32))
    dm = np.ones((128, 640), np.float32)
    dm[:, 0:128] = trif
    dm[:, 384:512] = trif
    dm[:, 512:640] = trif
    dm = dm.astype(bf16)
    in_maps = []
    for core in range(8):
        b, g = divmod(core, 4)
        xt32 = np.ascontiguousarray(x[b].T).astype(np.float32)
        xt_hi = xt32.astype(e4m3)
        xt_lo = (xt32 - xt_hi.astype(np.float32)).astype(e4m3)
        wq_ = split8(np.ascontiguousarray(w_qkv[128 * g:128 * (g + 1), :].T * (scale * 64.0)))
        wk_ = split8(np.ascontiguousarray(w_qkv[C + 128 * g:C + 128 * (g + 1), :].T) * 64.0)
        wv_ = split8(np.ascontiguousarray(w_qkv[2 * C + 128 * g:2 * C + 128 * (g + 1), :].T) * 64.0)
        wp_ = np.ascontiguousarray(w_proj[:, 128 * g:128 * (g + 1)].T).astype(bf16)
        in_maps.append({
            "xth": xt_hi, "xtl": xt_lo,
            "wq8": wq_, "wk8": wk_, "wv8": wv_,
            "wp": wp_, "tri": tri, "dmask": dm,
        })

    res = run_bass_kernel_spmd(
        nc,
        in_maps,
        core_ids=list(range(8)),
        trace=bool(os.environ.get("KERNEL_TRACE")),
    )
    LAST_RESULTS = res

    y = np.empty((B, N, C), np.float32)
    for b in range(B):
        acc = res.results[4 * b]["yt"].astype(np.float32)
        for g in range(1, 4):
            acc = acc + res.results[4 * b + g]["yt"]
        y[b] = acc.T + b_proj
    return y


# revision 5
# speedup vs baseline: 1.0132x; 1.0132x over previous
"""Multi-head causal self-attention (B=2, N=4096, C=512, H=8, D=64) on 8 TRN2 cores.

Sharding: core = b*4 + g  (b = batch 0..1, g = head-group 0..3, 2 heads each).

v2 restructure vs baseline:
- AV matmuls use P as the stationary operand: out' [q, v] per 128-q chunk,
  free dim 65 (64 v-dims + ones column for the softmax denominator) instead
  of 512 -> AV PE cycles halve.
- Softmax normalization becomes a per-partition scalar multiply (DVE), no
  gpsimd partition_broadcast.
- The [q, v] -> [v, q] layout flip for the projection uses the DMA XBAR
  transpose (SBUF->SBUF, off-engine).
- exp splits between ACT (exact, Exp activation) and DVE (Schraudolph:
  one tensor_scalar computing i16 = rint(A*s + B) whose bit pattern IS the
  bf16 of exp(s); ~2% deterministic wiggle, mostly cancelled by the
  consistent denominator).
- tri-mask multiplies run on Pool (gpsimd, SBUF-only engine).
"""

import os

import numpy as np
import ml_dtypes

_CACHE: dict = {}
LAST_RESULTS = None

B, C = 2, 512
H, D = 8, 64
N = 4096
NQT = 8          # q tiles of 512
NKB = 32         # key blocks of 128
QT = 512
KB = 128

# Schraudolph bf16-exp constants (i16 = rint(A*s + Bc), RNE convert verified)
SCH_A = 2.0**7 / float(np.log(2.0))
SCH_B = 127.0 * 2.0**7 - 0.0573 * 2.0**7

# fraction control: full-pair units with (index % SCH_MOD == SCH_PHASE) go to DVE
SCH_MOD = 3
SCH_PHASE = 1


def _build():
    import concourse.bass as bass
    import concourse.bacc as bacc
    import concourse.mybir as mybir
    import concourse.tile as tile

    dt = mybir.dt
    bf = dt.bfloat16
    f32 = dt.float32
    i16 = dt.int16
    Exp = mybir.ActivationFunctionType.Exp
    Copy = mybir.ActivationFunctionType.Copy
    Alu = mybir.AluOpType

    debug = bool(os.environ.get("KERNEL_DEBUG"))
    f8 = dt.float8e4
    DR = mybir.MatmulPerfMode.DoubleRow
    nc = bacc.Bacc("TRN2", target_bir_lowering=False)
    xth = nc.dram_tensor("xth", [C, N], f8, kind="ExternalInput")
    xtl = nc.dram_tensor("xtl", [C, N], f8, kind="ExternalInput")
    wq8 = nc.dram_tensor("wq8", [2, C, 128], f8, kind="ExternalInput")
    wk8 = nc.dram_tensor("wk8", [2, C, 128], f8, kind="ExternalInput")
    wv8 = nc.dram_tensor("wv8", [2, C, 128], f8, kind="ExternalInput")
    wp = nc.dram_tensor("wp", [128, C], bf, kind="ExternalInput")
    tri = nc.dram_tensor("tri", [128, 128], bf, kind="ExternalInput")
    dmask = nc.dram_tensor("dmask", [128, 640], bf, kind="ExternalInput")
    yt = nc.dram_tensor("yt", [C, N], f32, kind="ExternalOutput")
    if debug:
        d_qT = nc.dram_tensor("d_qT", [128, N], bf, kind="ExternalOutput")
        d_kT = nc.dram_tensor("d_kT", [128, N], bf, kind="ExternalOutput")
        d_v = nc.dram_tensor("d_v", [128, NKB, 130], bf, kind="ExternalOutput")
        d_pf = nc.dram_tensor("d_pf", [128, 1024], bf, kind="ExternalOutput")
        d_psO = nc.dram_tensor("d_psO", [128, 4, 128], f32, kind="ExternalOutput")
        d_on = nc.dram_tensor("d_on", [128, 4, 128], bf, kind="ExternalOutput")
        d_ot = nc.dram_tensor("d_ot", [128, 512], bf, kind="ExternalOutput")

    with tile.TileContext(nc) as tc:
        with (
            tc.tile_pool(name="persist", bufs=1) as pp,
            tc.tile_pool(name="pf", bufs=8) as pf_pool,      # P tiles (pairs)
            tc.tile_pool(name="pd", bufs=4) as pd_pool,      # P tiles (diag)
            tc.tile_pool(name="on", bufs=3) as on_pool,      # [q,v] normalized
            tc.tile_pool(name="ot", bufs=3) as ot_pool,      # transposed [v,q]
            tc.tile_pool(name="rc", bufs=3) as rc_pool,      # reciprocals
            tc.tile_pool(name="yo", bufs=4) as yo_pool,      # y staging
            tc.tile_pool(name="ps_s", bufs=3, space="PSUM") as ps_s,
            tc.tile_pool(name="ps_o", bufs=1, space="PSUM") as ps_o,
            tc.tile_pool(name="ps_y", bufs=1, space="PSUM") as ps_y,
        ):
            xt_hi = pp.tile([128, 4, N], f8)
            xt_lo = pp.tile([128, 4, N], f8)
            wq_sb = pp.tile([128, 2, 4, 128], f8)
            wk_sb = pp.tile([128, 2, 4, 128], f8)
            wv_sb = pp.tile([128, 2, 4, 128], f8)
            wp_sb = pp.tile([128, C], bf)
            tri_sb = pp.tile([128, 128], bf)
            qT = pp.tile([128, N], bf)
            kT = pp.tile([128, N], bf)
            v_sb = pp.tile([128, NKB, 130], bf)

            nc.gpsimd.dma_start(out=wq_sb[:, :, :, :], in_=wq8.rearrange("s (c p) f -> p s c f", p=128))
            nc.gpsimd.dma_start(out=wk_sb[:, :, :, :], in_=wk8.rearrange("s (c p) f -> p s c f", p=128))
            nc.gpsimd.dma_start(out=wv_sb[:, :, :, :], in_=wv8.rearrange("s (c p) f -> p s c f", p=128))
            nc.gpsimd.dma_start(out=wp_sb, in_=wp[:, :])
            nc.gpsimd.dma_start(out=tri_sb, in_=tri[:, :])
            dmask_sb = pp.tile([128, 640], bf)
            nc.gpsimd.dma_start(out=dmask_sb, in_=dmask[:, :])
            nc.vector.memset(v_sb, 1.0)
            # trigger the Exp act-table load early, overlapped with input DMAs
            warm = pp.tile([128, 1], f32)
            nc.vector.memset(warm, 0.0)
            nc.scalar.activation(warm, warm, Exp)

            xth_re = xth.rearrange("(c p) n -> p c n", p=128)
            xtl_re = xtl.rearrange("(c p) n -> p c n", p=128)
            UNSCALE = 1.0 / 64.0

            # -------- QKV phase: fp8 DoubleRow, 3-term hi/lo compensation --
            def pa_qk(n, dst, wsb, with_dma):
                def piece():
                    if with_dma:
                        nc.sync.dma_start(
                            out=xt_hi[:, :, QT * n:QT * (n + 1)],
                            in_=xth_re[:, :, QT * n:QT * (n + 1)],
                        )
                        nc.sync.dma_start(
                            out=xt_lo[:, :, QT * n:QT * (n + 1)],
                            in_=xtl_re[:, :, QT * n:QT * (n + 1)],
                        )
                    ps = ps_s.tile([128, 1024], f32, tag="s", name=f"pa_{n}")
                    pq = ps[:, 0:512]
                    terms = [(0, xt_hi), (0, xt_lo), (1, xt_hi)]
                    nmm = 0
                    for s, xt8 in terms:
                        for cp in range(2):
                            nc.tensor.matmul(
                                pq,
                                wsb[:, s, 2 * cp:2 * cp + 2, :],
                                xt8[:, 2 * cp:2 * cp + 2, QT * n:QT * (n + 1)],
                                start=(nmm == 0),
                                stop=(nmm == 5),
                                perf_mode=DR,
                            )
                            nmm += 1
                    nc.scalar.activation(dst[:, QT * n:QT * (n + 1)], pq, Copy, scale=UNSCALE)
                return piece

            def pa_v(n):
                # 4 kb blocks' V in one psum tile, one batched copy out
                def piece():
                    ps = ps_s.tile([128, 1024], f32, tag="s", name=f"pav_{n}")
                    for j in range(4):
                        kb = 4 * n + j
                        pv = ps[:, 128 * j:128 * (j + 1)]
                        terms = [(xt_hi, 0), (xt_lo, 0), (xt_hi, 1)]
                        nmm = 0
                        for xt8, s in terms:
                            for cp in range(2):
                                nc.tensor.matmul(
                                    pv,
                                    xt8[:, 2 * cp:2 * cp + 2, KB * kb:KB * (kb + 1)],
                                    wv_sb[:, s, 2 * cp:2 * cp + 2, :],
                                    start=(nmm == 0),
                                    stop=(nmm == 5),
                                    perf_mode=DR,
                                )
                                nmm += 1
                    nc.scalar.activation(
                        v_sb[:, 4 * n:4 * n + 4, :]
                        .rearrange("p k (h j) -> p k h j", h=2)[:, :, :, 0:64],
                        ps[:, 0:512].rearrange("p (k h j) -> p k h j", k=4, h=2),
                        Copy,
                        scale=UNSCALE,
                    )
                return piece

            def phase_a_pieces(n):
                return [
                    pa_qk(n, qT, wq_sb, True),
                    pa_qk(n, kT, wk_sb, False),
                    pa_v(n),
                ]

            # diag slot layout keeps every matmul inside one 2KB PSUM bank:
            # r1 -> [0:384], r3 -> [384:512] (bank 0), r2 -> [512:768] (bank 1)
            offs = (0, 512, 384)
            wid = (384, 256, 128)

            psO_map = {}
            on_map = {}
            ot_map = {}
            import heapq
            deferred = []
            seq_counter = [0]

            def defer(due, fn):
                heapq.heappush(deferred, (due, seq_counter[0], fn))
                seq_counter[0] += 1

            def flush(i):
                while deferred and deferred[0][0] <= i:
                    heapq.heappop(deferred)[2]()

            def get_psO(qt, h):
                key = (qt, h)
                if key not in psO_map:
                    psO_map[key] = ps_o.tile([128, 4, 128], f32, tag="o", name=f"psO_{qt}_{h}")
                return psO_map[key]

            def get_on(qt):
                if qt not in on_map:
                    on_map[qt] = on_pool.tile([128, 4, 128], bf, tag="on", name=f"on_{qt}")
                return on_map[qt]

            # --- AV' matmuls: P stationary [128 keys, 128 q], V moving [128,65]
            # PSUM has_written semantics: start=True clears the bits for the
            # WHOLE bank, so only the very first matmul of each (qt,h) stream
            # may set it. Later first-writes to other qc ranges overwrite
            # where the bit is unset, which is exactly what we need.
            def make_av(qt, h, contribs):
                # contribs: list of (kb, P_ap_slice_fn(qc) -> AP, qc_range)
                def av():
                    psO = get_psO(qt, h)
                    for kb, pap, qcs in contribs:
                        for qc in qcs:
                            nc.tensor.matmul(
                                psO[:, qc, 0:65],
                                pap(qc),
                                v_sb[:, kb, 65 * h:65 * h + 65],
                                start=(kb == 0 and qc == 0),
                                stop=(kb == 4 * qt + qc),
                                skip_group_check=True,
                            )
                return av

            def make_epilogue(qt, h):
                def epi():
                    psO = psO_map.pop((qt, h))
                    if debug and qt == 2 and h == 0:
                        tmp = rc_pool.tile([128, 4, 128], f32, tag="dbg")
                        nc.vector.tensor_copy(tmp, psO)
                        nc.sync.dma_start(out=d_psO[:, :, :], in_=tmp)
                    rc = rc_pool.tile([128, 4], f32, tag="rc")
                    nc.vector.reciprocal(out=rc, in_=psO[:, :, 64])
                    on = get_on(qt)
                    ot = None
                    if h == 1:
                        ot = ot_pool.tile([128, 512], bf, tag="ot", name=f"ot_{qt}")
                        ot_map[qt] = ot
                        if debug and qt == 2:
                            nc.sync.dma_start(out=d_on[:, :, :], in_=on)
                    for qc in range(4):
                        nc.vector.tensor_scalar(
                            out=on[:, qc, 64 * h:64 * h + 64],
                            in0=psO[:, qc, 0:64],
                            scalar1=rc[:, qc:qc + 1],
                            scalar2=None,
                            op0=mybir.AluOpType.mult,
                        )
                        if h == 1:
                            # transpose each chunk as soon as both halves exist
                            eng = nc.sync if qc % 2 == 0 else nc.scalar
                            eng.dma_start(
                                out=ot[:, 128 * qc:128 * (qc + 1)],
                                in_=on[:, qc, :],
                                transpose=True,
                            )
                    if h == 1:
                        on_map.pop(qt)
                        if debug and qt == 2:
                            nc.sync.dma_start(out=d_ot[:, :], in_=ot)
                return epi

            def make_proj_ob(qt, ob):
                def proj():
                    ot = ot_map[qt]
                    # the last q-tile's projections run in the drain; borrow
                    # the then-idle S pool for double buffering
                    pool = ps_s if qt == NQT - 1 else ps_y
                    tag = "s" if qt == NQT - 1 else "y"
                    psY = pool.tile([128, 512], f32, tag=tag, name=f"psY_{qt}_{ob}")
                    nc.tensor.matmul(
                        psY,
                        wp_sb[:, 128 * ob:128 * (ob + 1)],
                        ot,
                        start=True,
                        stop=True,
                    )
                    y_sb = yo_pool.tile([128, 512], f32, tag="yo")
                    if qt == NQT - 1 and ob % 2 == 1:
                        nc.scalar.activation(y_sb, psY, Copy)
                    else:
                        nc.vector.tensor_copy(y_sb, psY)
                    eng = nc.sync if (qt < NQT - 1 or ob % 2 == 0) else nc.scalar
                    eng.dma_start(
                        out=yt[128 * ob:128 * (ob + 1), QT * qt:QT * (qt + 1)],
                        in_=y_sb,
                    )
                    if ob == 3:
                        ot_map.pop(qt)
                return proj

            ui = 0
            for n in (0, 1):
                for piece in phase_a_pieces(n):
                    piece()
            pa_pending = []
            for qt in range(NQT):
                for piece in pa_pending:
                    piece()
                pa_pending = phase_a_pieces(qt + 2) if qt + 2 < NQT else []
                for h in range(2):
                    b0 = 64 * h
                    pair_idx = 0
                    # ---- full pair units: kb groups of 2 over kb = 0..4qt-1,
                    #      then singleton kb=4qt (block-diagonal, tri-masked)
                    kb = 0
                    while kb <= 4 * qt:
                        w = min(2, 4 * qt + 1 - kb)
                        kbs = list(range(kb, kb + w))
                        is_singleton = kbs[-1] == 4 * qt
                        psS = ps_s.tile([128, 1024], f32, tag="s")
                        for j, kbj in enumerate(kbs):
                            nc.tensor.matmul(
                                psS[:, 512 * j:512 * (j + 1)],
                                kT[b0:b0 + 64, KB * kbj:KB * (kbj + 1)],
                                qT[b0:b0 + 64, QT * qt:QT * (qt + 1)],
                                start=True,
                                stop=True,
                            )
                        Pf = pf_pool.tile([128, 1024], bf, tag="pf")
                        use_sch = (
                            not is_singleton
                            and pair_idx % SCH_MOD == SCH_PHASE
                        )
                        if use_sch:
                            nc.vector.tensor_scalar(
                                out=Pf[:, 0:512 * w].bitcast(i16),
                                in0=psS[:, 0:512 * w],
                                scalar1=SCH_A,
                                scalar2=SCH_B,
                                op0=Alu.mult,
                                op1=Alu.add,
                            )
                        else:
                            nc.scalar.activation(Pf[:, 0:512 * w], psS[:, 0:512 * w], Exp)
                        if is_singleton:
                            j = w - 1
                            nc.gpsimd.tensor_mul(
                                Pf[:, 512 * j:512 * j + 128],
                                Pf[:, 512 * j:512 * j + 128],
                                tri_sb,
                            )
                        if debug and qt == 2 and h == 0 and kb == 0:
                            nc.sync.dma_start(out=d_pf[:, :], in_=Pf)
                        # AV contributions of this unit (bind Pf NOW — late
                        # binding would capture a future unit's tile)
                        contribs = [
                            (kbj, (lambda jj, P: lambda qc: P[:, 512 * jj + 128 * qc:512 * jj + 128 * qc + 128])(j2, Pf), range(4))
                            for j2, kbj in enumerate(kbs)
                        ]
                        flush(ui)
                        last_stream = qt == NQT - 1 and h == 1
                        defer(ui + (1 if last_stream else 4), make_av(qt, h, contribs))
                        if pa_pending:
                            pa_pending.pop(0)()
                        ui += 1
                        kb += w
                        pair_idx += 1
                    # ---- diag unit: r = 1..3 packed [r1|r3|r2]
                    psD = ps_s.tile([128, 1024], f32, tag="s")
                    for r in (1, 2, 3):
                        kbr = 4 * qt + r
                        nc.tensor.matmul(
                            psD[:, offs[r - 1]:offs[r - 1] + wid[r - 1]],
                            kT[b0:b0 + 64, KB * kbr:KB * (kbr + 1)],
                            qT[b0:b0 + 64, QT * qt + 128 * r:QT * qt + 128 * r + wid[r - 1]],
                            start=True,
                            stop=True,
                        )
                    Pd = pd_pool.tile([128, 768], bf, tag="pd")
                    nc.vector.tensor_scalar(
                        out=Pd.bitcast(i16),
                        in0=psD[:, 0:768],
                        scalar1=SCH_A,
                        scalar2=SCH_B,
                        op0=Alu.mult,
                        op1=Alu.add,
                    )
                    nc.vector.tensor_mul(Pd[:, 0:640], Pd[:, 0:640], dmask_sb)
                    contribs = [
                        (4 * qt + r,
                         (lambda rr, P: lambda qc: P[:, offs[rr - 1] + 128 * (qc - rr):offs[rr - 1] + 128 * (qc - rr) + 128])(r, Pd),
                         range(r, 4))
                        for r in (1, 2, 3)
                    ]
                    flush(ui)
                    last_stream = qt == NQT - 1 and h == 1
                    defer(ui + (1 if last_stream else 3), make_av(qt, h, contribs))
                    defer(ui + (2 if last_stream else 4), make_epilogue(qt, h))
                    if h == 1:
                        if qt == NQT - 1:
                            for ob in range(4):
                                defer(ui + 3 + ob, make_proj_ob(qt, ob))
                        else:
                            # land the y-copies in the next qt's h0/h1
                            # boundary hole on ACT
                            for ob in range(4):
                                defer(ui + 5 + 2 * ob, make_proj_ob(qt, ob))
                    if pa_pending:
                        pa_pending.pop(0)()
                    ui += 1
            flush(10 ** 9)
            if debug:
                nc.sync.dma_start(out=d_qT[:, :], in_=qT)
                nc.sync.dma_start(out=d_kT[:, :], in_=kT)
                nc.sync.dma_start(out=d_v[:, :, :], in_=v_sb)

    nc.compile()
    return nc


def kernel(x, w_qkv, w_proj, b_proj):
    global LAST_RESULTS
    from concourse.bass_utils import run_bass_kernel_spmd

    if "nc" not in _CACHE:
        _CACHE["nc"] = _build()
    nc = _CACHE["nc"]

    x = np.asarray(x)
    w_qkv = np.asarray(w_qkv)
    w_proj = np.asarray(w_proj)
    b_proj = np.asarray(b_proj)
    bf16 = ml_dtypes.bfloat16
    e4m3 = ml_dtypes.float8_e4m3fn
    scale = D ** -0.5

    def split8(a):
        hi = a.astype(e4m3)
        lo = (a - hi.astype(np.float32)).astype(e4m3)
        return np.stack([hi, lo])

    tri = np.triu(np.ones((128, 128), np.float32)).astype(bf16)
    trif = np.triu(np.ones((128, 128), np.float32))
    dm = np.ones((128, 640), np.float32)
    dm[:, 0:128] = trif
    dm[:, 384:512] = trif
    dm[:, 512:640] = trif
    dm = dm.astype(bf16)
    in_maps = []
    for core in range(8):
        b, g = divmod(core, 4)
        xt32 = np.ascontiguousarray(x[b].T).astype(np.float32)
        xt_hi = xt32.astype(e4m3)
        xt_lo = (xt32 - xt_hi.astype(np.float32)).astype(e4m3)
        wq_ = split8(np.ascontiguousarray(w_qkv[128 * g:128 * (g + 1), :].T * (scale * 64.0)))
        wk_ = split8(np.ascontiguousarray(w_qkv[C + 128 * g:C + 128 * (g + 1), :].T) * 64.0)
        wv_ = split8(np.ascontiguousarray(w_qkv[2 * C + 128 * g:2 * C + 128 * (g + 1), :].T) * 64.0)
        wp_ = np.ascontiguousarray(w_proj[:, 128 * g:128 * (g + 1)].T).astype(bf16)
        in_maps.append({
            "xth": xt_hi, "xtl": xt_lo,
            "wq8": wq_, "wk8": wk_, "wv8": wv_,
            "wp": wp_, "tri": tri, "dmask": dm,
        })

    res = run_bass_kernel_spmd(
        nc,
        in_maps,
        core_ids=list(range(8)),
        trace=bool(os.environ.get("KERNEL_TRACE")),
    )
    LAST_RESULTS = res

    y = np.empty((B, N, C), np.float32)
    for b in range(B):
        acc = res.results[4 * b]["yt"].astype(np.float32)
        for g in range(1, 4):
            acc = acc + res.results[4 * b + g]["yt"]
        y[b] = acc.T + b_proj
    return y
